# revision 28
# baseline (speedup 1.0000x reference)
"""Trainium2 Bass kernel for nn_Attention_66563403153646.

Dense transformer attention block with rotary embeddings + gated adapter
(prefix) attention, fp32 reference:

    y = softmax(rope(x@wq) @ rope(x@wk).T * k + mask) @ (x@wv)
      + gate * softmax(rope(x@wq) @ (adapter@wk).T * k) @ (adapter@wv)
    out = y @ wo

Sharding: 4-way tensor-parallel over heads x 2-way data-parallel over batch
(8 NeuronCores). Each core computes a [S, D] partial of its batch's output
(its 8 heads' contribution through wo); the host sums the 4 TP partials.

v2 design (PE-bound; keep the tensor engine streaming at 2.4 GHz):
  - All matmul operands bf16 (same PE rate as f32r at moving dim >= 256,
    half the DMA/SBUF). Host pre-converts and pre-lays-out weights so
    every DMA is contiguous; host also precomputes the tiny adapter
    projections ak/av (84 MFLOP) so no device cycles go to them.
  - x is staged [P, DC, S] and streamed in 4 s-passes of 512 with
    grouped chunk DMAs and a double-buffered tile; the first head's
    wq/wk prefetch on a separate queue so the PE starts early.
  - q/k are computed per-head in [HD, S] layout with rope-pair-permuted
    head dims (host permutes wq/wk columns) so RoPE is two
    partition-halves of elementwise ops. q/k/v round-trip DRAM in bf16.
  - Phase 2 runs a 3-stage software pipeline per q-tile of 512:
    A: scores ([k,q] layout, causal blocks only) + exp -> ptb (bf16)
    B: column sums (ones-matmul) + approx-reciprocal rows
    C: broadcast-normalize (K=1 f32r matmul) + p@v + adapter pv
       accumulated into the same PSUM group (pre-scaled by
       gate*smain/asum so one normalization covers both terms)
    scheduled A(Q) between B(Q-1) and C(Q-1), with per-head tails
    deferred past the next head's preamble, so the PE never waits on
    the DVE/ACT row ops ([1,N] DVE ops are single-lane and slow --
    plain reciprocal on [1,2048] costs 12.9us; approx_fast + copies
    spread across vector/scalar keep every queue shallow).
  - Attention outputs accumulate into an SBUF-resident [HD, 8, S] bf16
    tile that is exactly the lhsT layout the final wo matmul needs: no
    oT DRAM round trip. Phase 3 shares the phase-2 pool scope (PSUM
    pools included) so wo prefetches during attention and the
    transition costs nothing.
"""

import sys

sys.path.insert(0, "/opt/trn_rl_repo")

import math
from dataclasses import dataclass

import numpy as np

import concourse.bass as bass
import concourse.mybir as mybir
import concourse.tile as tile
from concourse import bacc
from concourse.masks import make_identity

f32 = mybir.dt.float32
f32r = mybir.dt.float32r
bf16 = mybir.dt.bfloat16

P = 128


@dataclass(frozen=True)
class Cfg:
    S: int = 2048  # sequence length
    D: int = 4096  # model dim
    HPC: int = 8  # heads per core
    HD: int = 128  # head dim
    AL: int = 10  # adapter len
    XS: int = 512  # phase-1 s-pass width

    @property
    def DC(self):  # D chunks of 128 (contraction)
        return self.D // P

    @property
    def NQ(self):  # 128-row q blocks
        return self.S // P

    @property
    def NT(self):  # 512-col tiles
        return self.S // 512

    @property
    def NXS(self):  # phase-1 passes
        return self.S // self.XS

    @property
    def NE(self):  # wo 512-col chunks
        return self.D // 512


def build_nc(cfg: Cfg):
    nc = bacc.Bacc(None, target_bir_lowering=False, debug=False)
    S, D, HPC, HD, AL, XS = cfg.S, cfg.D, cfg.HPC, cfg.HD, cfg.AL, cfg.XS
    DC, NQ, NT, NXS, NE = cfg.DC, cfg.NQ, cfg.NT, cfg.NXS, cfg.NE
    HH = HD // 2
    inv_sqrt = 1.0 / math.sqrt(HD)

    # ---- I/O (host pre-layouts; all contiguous DMA) ----
    x_d = nc.dram_tensor("xp", [P, DC, S], bf16, kind="ExternalInput")
    wq_d = nc.dram_tensor("wq", [HPC, P, DC, HD], bf16, kind="ExternalInput")
    wk_d = nc.dram_tensor("wk", [HPC, P, DC, HD], bf16, kind="ExternalInput")
    wv_d = nc.dram_tensor("wv", [HPC, P, DC, HD], bf16, kind="ExternalInput")
    wo_d = nc.dram_tensor("wo", [NE, P, HPC, 512], bf16, kind="ExternalInput")
    cosT_d = nc.dram_tensor("cosT", [HH, S], f32, kind="ExternalInput")
    sinT_d = nc.dram_tensor("sinT", [HH, S], f32, kind="ExternalInput")
    # 0/1 upper-triangular (incl diag) [k,q] mask for the diagonal block
    tri_d = nc.dram_tensor("tri", [P, P], bf16, kind="ExternalInput")
    gate_d = nc.dram_tensor("gater", [1, HPC], f32, kind="ExternalInput")
    akT_d = nc.dram_tensor("akT", [HPC, P, AL], bf16, kind="ExternalInput")
    av_d = nc.dram_tensor("av", [HPC, AL, P], bf16, kind="ExternalInput")
    y_d = nc.dram_tensor("y", [S, D], f32, kind="ExternalOutput")

    ExpF = mybir.ActivationFunctionType.Exp
    Mul = mybir.AluOpType.mult

    with tile.TileContext(nc) as tc:
        with (
            tc.tile_pool(name="persist", bufs=1) as persist,
            tc.tile_pool(name="dram", bufs=1, space="DRAM") as dram,
            tc.tile_pool(name="p2qkv", bufs=2) as p2qkv,
        ):
            # persistent small tiles (cos on partitions 0:64, sin on 64:128)
            cs_sb = persist.tile([P, S], f32)
            nc.scalar.dma_start(cs_sb[0:HH, :], cosT_d[:])
            nc.scalar.dma_start(cs_sb[HH:, :], sinT_d[:])
            tri_sb = persist.tile([P, P], bf16)
            nc.scalar.dma_start(tri_sb[:], tri_d[:])
            g_row = persist.tile([1, HPC], f32)
            nc.scalar.dma_start(g_row[:], gate_d[:])
            ident_f = persist.tile([P, P], f32)
            make_identity(nc, ident_f)
            ident = persist.tile([P, P], bf16)
            nc.vector.tensor_copy(ident[:], ident_f[:])
            ones_col = persist.tile([P, 1], bf16)
            nc.vector.memset(ones_col[:], 1.0)
            ones_colA = persist.tile([AL, 1], bf16)
            nc.vector.memset(ones_colA[:], 1.0)
            ones_rowA = persist.tile([1, AL], bf16)
            nc.vector.memset(ones_rowA[:], 1.0)
            ones_row_f = persist.tile([1, P], f32)
            nc.vector.memset(ones_row_f[:], 1.0)
            ones_row_r = persist.tile([1, P], f32r)
            nc.vector.tensor_copy(ones_row_r[:], ones_row_f[:])
            # attention output accumulator: [HD, head, S] == wo lhsT layout
            oT_all = persist.tile([P, HPC, S], bf16)

            # DRAM scratch (bf16 round trip for q/k/v)
            qT_dr = dram.tile([HPC, P, S], bf16)
            kT_dr = dram.tile([HPC, P, S], bf16)
            vT_dr = dram.tile([HPC, P, S], bf16)

            # ================= Phase 1: projections + rope =================
            with (
                tc.tile_pool(name="p1x", bufs=2) as p1x,
                tc.tile_pool(name="p1w", bufs=2) as p1w,
                tc.tile_pool(name="p1o", bufs=3) as p1o,
                tc.tile_pool(name="p1t", bufs=1) as p1t,
                tc.tile_pool(name="p1ps", bufs=4, space="PSUM") as p1ps,
            ):
                wt_pre = {}
                for proj, w_dram in (("q", wq_d), ("k", wk_d)):
                    wt = p1w.tile([P, DC, HD], bf16, tag="wt")
                    nc.gpsimd.dma_start(wt[:], w_dram[0])
                    wt_pre[proj] = wt
                for st in range(NXS):
                    soff = st * XS
                    xt = p1x.tile([P, DC, XS], bf16, tag="xt")
                    for gi, c in enumerate(range(0, DC, 4)):
                        eng = nc.scalar if (st == 0 and gi % 2) else nc.sync
                        eng.dma_start(
                            xt[:, c : c + 4, :],
                            x_d[:, c : c + 4, soff : soff + XS],
                        )
                    for h in range(HPC):
                        for proj, w_dram, out_dr in (
                            ("q", wq_d, qT_dr),
                            ("k", wk_d, kT_dr),
                            ("v", wv_d, vT_dr),
                        ):
                            if st == 0 and h == 0 and proj in wt_pre:
                                wt = wt_pre.pop(proj)
                            else:
                                wt = p1w.tile([P, DC, HD], bf16, tag="wt")
                                nc.sync.dma_start(wt[:], w_dram[h])
                            psum = p1ps.tile([P, XS], f32, tag="psum")
                            for c in range(DC):
                                nc.tensor.matmul(
                                    psum[:],
                                    wt[:, c, :],
                                    xt[:, c, :],
                                    start=(c == 0),
                                    stop=(c == DC - 1),
                                )
                            if proj == "v":
                                vt_sb = p1o.tile([P, XS], bf16, tag="o")
                                nc.scalar.copy(vt_sb[:], psum[:])
                                nc.gpsimd.dma_start(
                                    out_dr[h, :, soff : soff + XS], vt_sb[:]
                                )
                            else:
                                # rope: psum partitions 0:64 = even rope dims
                                # (x0), 64:128 = odd (x1). Products go to
                                # base-0 tmp tiles (PSUM x SBUF inputs may
                                # differ in base partition), combines are
                                # base-aligned.
                                c_ap = cs_sb[0:HH, soff : soff + XS]
                                s_ap = cs_sb[HH:, soff : soff + XS]
                                x0 = psum[0:HH, :]
                                x1 = psum[HH : 2 * HH, :]
                                ta = p1t.tile([HH, XS], f32, tag="ta")
                                tb = p1t.tile([HH, XS], f32, tag="tb")
                                tc2 = p1t.tile([HH, XS], f32, tag="tc")
                                td = p1t.tile([HH, XS], f32, tag="td")
                                qt_sb = p1o.tile([P, XS], bf16, tag="o")
                                nc.vector.tensor_tensor(ta[:], x0, c_ap, op=Mul)
                                nc.vector.tensor_tensor(tb[:], x1, s_ap, op=Mul)
                                nc.vector.tensor_sub(qt_sb[0:HH, :], ta[:], tb[:])
                                nc.vector.tensor_tensor(tc2[:], x0, s_ap, op=Mul)
                                nc.vector.tensor_tensor(td[:], x1, c_ap, op=Mul)
                                nc.vector.tensor_add(qt_sb[HH:, :], tc2[:], td[:])
                                nc.gpsimd.dma_start(
                                    out_dr[h, :, soff : soff + XS], qt_sb[:]
                                )

            # ================= Phase 2: attention per head =================
            # scoresT layout [k, q]: p = exp(kT_blk.T @ qT_tile * inv_sqrt)
            # lands directly in the layout p@v needs. Scores are O(5) so exp
            # needs no max subtraction; causal masking multiplies the
            # diagonal-band blocks by a 0/1 mask; per-q sums come from a
            # ones-column matmul; normalization happens at eviction via a
            # K=1 broadcast matmul of 1/sums (f32r, exact).
            with (
                tc.tile_pool(name="p2vn", bufs=2) as p2vn,
                tc.tile_pool(name="p2pt", bufs=2) as p2pt,
                tc.tile_pool(name="p2sm", bufs=2) as p2sm,
                tc.tile_pool(name="p2row", bufs=3) as p2row,
                tc.tile_pool(name="p2hrow", bufs=2) as p2hrow,
                tc.tile_pool(name="p2ps_s", bufs=2, space="PSUM") as p2ps_s,
                tc.tile_pool(name="p2ps_o", bufs=2, space="PSUM") as p2ps_o,
                tc.tile_pool(name="p2ps_c", bufs=2, space="PSUM") as p2ps_c,
                tc.tile_pool(name="p2ps_u", bufs=2, space="PSUM") as p2ps_u,
                tc.tile_pool(name="p3w", bufs=2) as p3w,
                tc.tile_pool(name="p3y", bufs=3) as p3y,
            ):

                def stage_A(h, Q, qT, kT, ptb, jb_lo, jb_hi):
                    """scores + exp -> ptb (bf16), causal blocks only."""
                    for jb in range(jb_lo, jb_hi):
                        off = 0 if jb < 4 * Q else (jb - 4 * Q) * P
                        ps_s = p2ps_s.tile([P, 512], f32, tag="s")
                        nc.tensor.matmul(
                            ps_s[:, : 512 - off],
                            kT[:, jb * P : (jb + 1) * P],
                            qT[:, Q * 512 + off : (Q + 1) * 512],
                            start=True,
                            stop=True,
                        )
                        nc.scalar.activation(
                            ptb[:, jb, off:],
                            ps_s[:, : 512 - off],
                            ExpF,
                            bias=0.0,
                            scale=inv_sqrt,
                        )
                        if jb >= 4 * Q:
                            # diagonal block: 0/1 triangle on the aligned
                            # [128,128] sub-block
                            nc.gpsimd.tensor_tensor(
                                ptb[:, jb, off : off + P],
                                ptb[:, jb, off : off + P],
                                tri_sb[:],
                                op=Mul,
                            )

                def stage_B(h, Q, ptb, garow):
                    """column sums -> 1/sums row + adapter scale row."""
                    nkb = (Q + 1) * 4
                    ps_su = p2ps_u.tile([1, 512], f32, tag="u")
                    for jb in range(nkb):
                        off = 0 if jb < 4 * Q else (jb - 4 * Q) * P
                        nc.tensor.matmul(
                            ps_su[:, off:],
                            ones_col[:],
                            ptb[:, jb, off:],
                            start=(jb == 0),
                            stop=(jb == nkb - 1),
                        )
                    # crow = smain * gate/asum  (pre-scale for adapter pv so
                    # the final 1/smain normalization covers both terms)
                    crow = p2row.tile([1, 512], bf16, tag="crow")
                    nc.vector.tensor_tensor(
                        crow[:], ps_su[:], garow[:, Q * 512 : (Q + 1) * 512], op=Mul
                    )
                    rrow_f = p2row.tile([1, 512], f32, tag="rrowf")
                    # sums are well inside [1e-3, 1e4]: approx (18 bits) is
                    # plenty for a softmax normalization, ~5x cheaper on DVE
                    nc.vector.reciprocal_approx_fast(rrow_f[:], ps_su[:])
                    rrow = p2row.tile([1, 512], f32r, tag="rrow")
                    nc.vector.tensor_copy(rrow[:], rrow_f[:])
                    return rrow, crow

                def stage_C(h, Q, ptb, rrow, crow, v_nat, av, exp_aT):
                    """p@v + broadcast-normalize + adapter pv + evict.

                    The pv matmuls need nothing from stage B, so they run
                    first and cover the DVE latency of the rrow/crow chain
                    before the broadcast matmuls consume those rows."""
                    nkb = (Q + 1) * 4
                    ps_o = p2ps_o.tile([P, 512], f32, tag="o")
                    for jb in range(nkb):
                        off = 0 if jb < 4 * Q else (jb - 4 * Q) * P
                        nc.tensor.matmul(
                            ps_o[:, off:],
                            v_nat[:, jb, :],
                            ptb[:, jb, off:],
                            start=(jb == 0),
                            stop=False,
                        )
                    ps_b10 = p2ps_c.tile([AL, 512], f32, tag="c")
                    nc.tensor.matmul(
                        ps_b10[:], ones_rowA[:], crow[:], start=True, stop=True
                    )
                    ps_bc = p2ps_c.tile([P, 512], f32, tag="c")
                    nc.tensor.matmul(
                        ps_bc[:],
                        ones_row_r[:],
                        rrow[:],
                        start=True,
                        stop=True,
                    )
                    b10_sb = p2row.tile([AL, 512], bf16, tag="b10")
                    nc.vector.tensor_copy(b10_sb[:], ps_b10[:])
                    ap_s = p2sm.tile([AL, 512], bf16, tag="aps")
                    nc.vector.tensor_tensor(
                        ap_s[:],
                        exp_aT[:, Q * 512 : (Q + 1) * 512],
                        b10_sb[:],
                        op=Mul,
                    )
                    nc.tensor.matmul(ps_o[:], av[:], ap_s[:], start=False, stop=True)
                    bc_sb = p2sm.tile([P, 512], f32, tag="bc")
                    nc.vector.tensor_copy(bc_sb[:], ps_bc[:])
                    nc.vector.tensor_tensor(
                        oT_all[:, h, Q * 512 : (Q + 1) * 512],
                        ps_o[:],
                        bc_sb[:],
                        op=Mul,
                    )

                # prefetch the first wo chunk during phase 2 (its DMA has
                # no dependencies; sync queue issues it right away)
                wo_first = p3w.tile([P, HPC, 512], bf16, tag="wo")
                nc.sync.dma_start(wo_first[:], wo_d[0])

                prev_tail = None
                for h in range(HPC):
                    qT = p2qkv.tile([P, S], bf16, tag="qT")
                    kT = p2qkv.tile([P, S], bf16, tag="kT")
                    vT = p2qkv.tile([P, S], bf16, tag="vT")
                    nc.scalar.dma_start(qT[:], qT_dr[h])
                    nc.scalar.dma_start(kT[:], kT_dr[h])
                    nc.scalar.dma_start(vT[:], vT_dr[h])
                    akT = p2sm.tile([P, AL], bf16, tag="akT")
                    av = p2sm.tile([AL, P], bf16, tag="av")
                    nc.scalar.dma_start(akT[:], akT_d[h])
                    nc.scalar.dma_start(av[:], av_d[h])

                    # preamble: v natural layout [s-block, NQ, d].  Runs
                    # before the previous head's tail so the ACT queue can
                    # drain that head's last exps first.
                    v_nat = p2vn.tile([P, NQ, P], bf16, tag="v_nat")
                    for i in range(NQ):
                        ps_vt = p2ps_c.tile([P, P], bf16, tag="c")
                        nc.tensor.transpose(
                            ps_vt[:], vT[:, i * P : (i + 1) * P], ident[:]
                        )
                        nc.scalar.copy(v_nat[:, i, :], ps_vt[:])


                    # preamble: adapter scores for the whole head in [AL, S]
                    exp_aT = p2sm.tile([AL, S], bf16, tag="expa")
                    asum_row = p2hrow.tile([1, S], f32, tag="asum")
                    for tq in range(NT):
                        sl = slice(tq * 512, (tq + 1) * 512)
                        pa = p2ps_c.tile([AL, 512], f32, tag="c")
                        nc.tensor.matmul(
                            pa[:], akT[:], qT[:, sl], start=True, stop=True
                        )
                        nc.scalar.activation(
                            exp_aT[:, sl], pa[:], ExpF, bias=0.0, scale=inv_sqrt
                        )
                        ps_as = p2ps_u.tile([1, 512], f32, tag="u")
                        nc.tensor.matmul(
                            ps_as[:], ones_colA[:], exp_aT[:, sl], start=True,
                            stop=True,
                        )
                        nc.vector.tensor_copy(asum_row[:, sl], ps_as[:])
                    garow = p2hrow.tile([1, S], f32, tag="garow")
                    nc.vector.reciprocal_approx_fast(garow[:], asum_row[:])
                    nc.vector.tensor_tensor(
                        garow[:],
                        garow[:],
                        g_row[0:1, h : h + 1].to_broadcast([1, S]),
                        op=Mul,
                    )

                    if prev_tail is not None:
                        # deferred tail of the previous head: by now its
                        # trailing exps have had the transposes + adapter
                        # preamble to drain, and stage C's leading pv block
                        # covers the reciprocal-row latency
                        tail_B, tail_C = prev_tail
                        tail_B()
                        tail_C()
                        prev_tail = None

                    prev = None
                    for Q in range(NT):
                        nkb = (Q + 1) * 4
                        ptb = p2pt.tile([P, NQ, 512], bf16, tag="ptb")
                        # the full scores pass of tile Q gives the previous
                        # tile's trailing exps maximum time to drain before
                        # its sums run; stage C's leading pv matmuls then
                        # cover the reciprocal-row DVE latency
                        stage_A(h, Q, qT, kT, ptb, 0, nkb)
                        if prev is not None:
                            pQ, pptb = prev
                            rrow, crow = stage_B(h, pQ, pptb, garow)
                            stage_C(h, pQ, pptb, rrow, crow, v_nat, av, exp_aT)
                        prev = (Q, ptb)

                    # tail for Q=NT-1, deferred into the next head's preamble
                    def mk_tail(h=h, prev=prev, garow=garow, v_nat=v_nat,
                                av=av, exp_aT=exp_aT):
                        state = {}

                        def tail_B():
                            state["rc"] = stage_B(h, prev[0], prev[1], garow)

                        def tail_C():
                            rrow, crow = state["rc"]
                            stage_C(h, prev[0], prev[1], rrow, crow, v_nat,
                                    av, exp_aT)

                        return tail_B, tail_C

                    prev_tail = mk_tail()

                # last head's tail
                tail_B, tail_C = prev_tail
                tail_B()
                tail_C()

                # ================= Phase 3: out @ wo =================
                for et in range(NE):
                    if et == 0:
                        wo_t = wo_first
                    else:
                        wo_t = p3w.tile([P, HPC, 512], bf16, tag="wo")
                        nc.sync.dma_start(wo_t[:], wo_d[et])
                    for st in range(NQ):
                        ps_pool = p2ps_s if st % 2 == 0 else p2ps_o
                        ps_y = ps_pool.tile(
                            [P, 512], f32, tag="s" if st % 2 == 0 else "o"
                        )
                        for h in range(HPC):
                            nc.tensor.matmul(
                                ps_y[:],
                                oT_all[:, h, st * P : (st + 1) * P],
                                wo_t[:, h, :],
                                start=(h == 0),
                                stop=(h == HPC - 1),
                            )
                        y_sb = p3y.tile([P, 512], f32, tag="ysb")
                        if st % 2 == 0:
                            nc.scalar.copy(y_sb[:], ps_y[:])
                        else:
                            nc.vector.tensor_copy(y_sb[:], ps_y[:])
                        nc.gpsimd.dma_start(
                            y_d[st * P : (st + 1) * P, et * 512 : (et + 1) * 512],
                            y_sb[:],
                        )

    nc.compile()
    return nc


# ====================== host side: sharding + runner ======================

B, S, D, H = 2, 2048, 4096, 32
HD = D // H
AL = 10
N_CORES = 8
TP = 4  # head groups
HPC = H // TP  # 8 heads per core
DC = D // P
NE = D // 512

_RUNNER = None


def _make_runner(nc, n_cores=N_CORES):
    import jax
    from jax.sharding import Mesh, PartitionSpec
    from jax.experimental.shard_map import shard_map

    from concourse import bass2jax
    from concourse.bass2jax import _bass_exec_p, install_neuronx_cc_hook

    install_neuronx_cc_hook()
    partition_name = nc.partition_id_tensor.name if nc.partition_id_tensor else None

    in_names, out_names, out_avals = [], [], []
    for alloc in nc.m.functions[0].allocations:
        if not isinstance(alloc, mybir.MemoryLocationSet):
            continue
        name = alloc.memorylocations[0].name
        if alloc.kind == "ExternalInput":
            if name != partition_name:
                in_names.append(name)
        elif alloc.kind == "ExternalOutput":
            out_names.append(name)
            out_avals.append(
                jax.core.ShapedArray(
                    tuple(alloc.tensor_shape), mybir.dt.np(alloc.dtype)
                )
            )
    n_params = len(in_names)
    n_outs = len(out_avals)
    all_in_names = list(in_names) + list(out_names)
    if partition_name is not None:
        all_in_names.append(partition_name)

    def _body(*args):
        operands = list(args)
        if partition_name is not None:
            operands.append(bass2jax.partition_id_tensor())
        outs = _bass_exec_p.bind(
            *operands,
            out_avals=tuple(out_avals),
            in_names=tuple(all_in_names),
            out_names=tuple(out_names),
            lowering_input_output_aliases=(),
            sim_require_finite=True,
            sim_require_nnan=True,
            nc=nc,
        )
        return tuple(outs)

    devices = jax.devices()[:n_cores]
    mesh = Mesh(np.asarray(devices), ("core",))
    fn = jax.jit(
        shard_map(
            _body,
            mesh=mesh,
            in_specs=(PartitionSpec("core"),) * (n_params + n_outs),
            out_specs=(PartitionSpec("core"),) * n_outs,
            check_rep=False,
        ),
        keep_unused=True,
    )

    class Runner:
        in_names_ = in_names
        out_names_ = out_names
        fn_ = fn

        def prep(self, in_maps):
            import jax as _jax

            concat_in = [
                np.concatenate(
                    [np.ascontiguousarray(in_maps[c][n]) for c in range(n_cores)],
                    axis=0,
                )
                for n in in_names
            ]
            concat_zero = [
                np.zeros((n_cores * a.shape[0], *a.shape[1:]), a.dtype)
                for a in out_avals
            ]
            shardings = [
                _jax.sharding.NamedSharding(mesh, PartitionSpec("core"))
            ] * (n_params + n_outs)
            return _jax.device_put(concat_in + concat_zero, shardings)

        def run(self, args):
            import jax as _jax

            outs = fn(*args)
            _jax.block_until_ready(outs)
            return [
                {
                    n: np.asarray(outs[i]).reshape(n_cores, *out_avals[i].shape)[c]
                    for i, n in enumerate(out_names)
                }
                for c in range(n_cores)
            ]

        def time_pipelined(self, args, reps=10, warmup=1):
            import time as _time

            import jax as _jax

            for _ in range(warmup):
                _jax.block_until_ready(fn(*args))
            t0 = _time.perf_counter()
            outs = None
            for _ in range(reps):
                outs = fn(*args)
            _jax.block_until_ready(outs)
            return (_time.perf_counter() - t0) / reps

    return Runner()


def _shard_inputs(x, cos, sin, mask, wq, wk, wv, wo, gate, adapter):
    """Build the 8 per-core input maps (bf16 pre-layouts)."""
    import ml_dtypes

    bf = ml_dtypes.bfloat16

    # rope permutation of head-dim columns: even dims first, odd second
    perm = np.concatenate([np.arange(0, HD, 2), np.arange(1, HD, 2)])
    col_perm = np.concatenate([h * HD + perm for h in range(H)])
    wq_p = np.asarray(wq, dtype=np.float32)[:, col_perm]
    wk_p = np.asarray(wk, dtype=np.float32)[:, col_perm]
    wv_f = np.asarray(wv, dtype=np.float32)
    wo_f = np.asarray(wo, dtype=np.float32)

    def w_heads(w_slice):
        # [D, HPC*HD] -> [HPC, P, DC, HD] bf16
        a = w_slice.reshape(DC, P, HPC, HD)
        return np.ascontiguousarray(a.transpose(2, 1, 0, 3)).astype(bf)

    cosT = np.ascontiguousarray(np.asarray(cos, np.float32).T)  # [64, S]
    sinT = np.ascontiguousarray(np.asarray(sin, np.float32).T)
    ad = np.asarray(adapter, np.float32)[0]  # [AL, D]
    ak_full = ad @ wk_p  # [AL, D] (rope-permuted cols, matching q layout)
    av_full = ad @ wv_f  # [AL, D]

    # 0/1 [k, q] allowed-mask of an aligned 128x128 diagonal block
    m = np.asarray(mask, dtype=np.float32)[0, 0]
    tri = np.ascontiguousarray((m[:P, :P].T == 0)).astype(bf)

    gate_v = np.asarray(gate, dtype=np.float32).reshape(H)

    xp = []
    for b in range(B):
        xT = np.asarray(x[b], np.float32).T  # [D, S]
        xp.append(
            np.ascontiguousarray(xT.reshape(DC, P, S).transpose(1, 0, 2)).astype(bf)
        )

    in_maps = []
    for c in range(N_CORES):
        b = c // TP
        g = c % TP
        hs = g * HPC * HD  # column slice start
        wo_slice = wo_f[hs : hs + HPC * HD, :]  # [DH, D]
        wo_prep = np.ascontiguousarray(
            wo_slice.reshape(HPC, P, NE, 512).transpose(2, 1, 0, 3)
        ).astype(bf)  # [NE, P, HPC, 512] bf16
        in_maps.append(
            {
                "xp": xp[b],
                "wq": w_heads(wq_p[:, hs : hs + HPC * HD]),
                "wk": w_heads(wk_p[:, hs : hs + HPC * HD]),
                "wv": w_heads(wv_f[:, hs : hs + HPC * HD]),
                "wo": wo_prep,
                "akT": np.ascontiguousarray(
                    ak_full[:, hs : hs + HPC * HD]
                    .reshape(AL, HPC, P)
                    .transpose(1, 2, 0)
                ).astype(bf),
                "av": np.ascontiguousarray(
                    av_full[:, hs : hs + HPC * HD]
                    .reshape(AL, HPC, P)
                    .transpose(1, 0, 2)
                ).astype(bf),
                "cosT": cosT,
                "sinT": sinT,
                "tri": tri,
                "gater": np.ascontiguousarray(
                    gate_v[g * HPC : (g + 1) * HPC][None, :]
                ),
            }
        )
    return in_maps


def get_runner():
    global _RUNNER
    if _RUNNER is None:
        nc = build_nc(Cfg())
        _RUNNER = _make_runner(nc)
    return _RUNNER


def kernel(**inputs) -> np.ndarray:
    x = np.asarray(inputs["x"])
    in_maps = _shard_inputs(
        x,
        inputs["cos"],
        inputs["sin"],
        inputs["mask"],
        inputs["wq"],
        inputs["wk"],
        inputs["wv"],
        inputs["wo"],
        inputs["gate"],
        inputs["adapter"],
    )
    runner = get_runner()
    args = runner.prep(in_maps)
    outs = runner.run(args)
    y = np.zeros((B, S, D), dtype=np.float32)
    for c in range(N_CORES):
        y[c // TP] += outs[c]["y"]
    return y
